# revision 37
# baseline (speedup 1.0000x reference)
"""ChunkedLinearAttention Trainium2 kernel — 8-core SPMD.

Sharding: core c -> batch b = c//2, head-half hh = c%2 (8 of 16 heads).
Each core computes qkv projection for its heads, chunked local attention +
cross-chunk linear term, and a row-sharded out-projection producing a partial
[4096, 1024] bf16 output; host sums the two half partials per batch element.

All matmuls in bf16 (fp32 accumulate in PSUM).  Layouts:
  xT    [1024, 4096]  x[b] transposed (host-side), bf16
  qkT   [cols, tok]   computed on PE: lhsT=Wqk tile, rhs=xT tile
  v     [tok, vcols]  computed on PE: lhsT=xT tile, rhs=Wv
  per head-pair: qT [128(2 heads x 64 dims), 8 chunks, 64 tok]
                 kT [128, 8 chunks, 65]  (col 64 = exclusive cum_k mean)
  scores S [128(2 heads x 64 q), 4, 65] x2 via per-chunk matmuls packed with
  tile_position; col 64 gives cross_pre = q . cum_k for free.
  sigmoid(cross_pre) is recovered from exp values: E64/(E64+1) on vector.
  out_localT [128(2 heads x 64 dims), 512 tok] accumulated in PSUM; the cross
  term is added with per-chunk rank-1 matmuls cumv[c] (x) crossT[c].

Pipeline (per emission iteration t):
  stage_a(t)   : projections + chunk means + cumsums (k and v)
  stage_b1(t-1): in-chunk score matmuls + exp
  stage_c(t-2) : out projection + store   (PE cover for b2's vector chain)
  stage_b2(t-1): softmax normalize + attn transpose + out_local + cross
"""

import sys

if "/opt/trn_rl_repo" not in sys.path:
    sys.path.insert(0, "/opt/trn_rl_repo")

import numpy as np
import ml_dtypes

import concourse.bacc as bacc
import concourse.tile as tile
import concourse.mybir as mybir
from concourse.bass_utils import run_bass_kernel_spmd

F32 = mybir.dt.float32
BF16 = mybir.dt.bfloat16
AFT = mybir.ActivationFunctionType
ALU = mybir.AluOpType

DIM, H, D, CS = 1024, 16, 64, 64
SCALE = D ** -0.5
B, N = 4, 4096
NBLK, TB = 8, 512          # token blocks
NC_CHUNKS = 8              # chunks per block
HPC = 8                    # heads per core
NPAIR = 4                  # head pairs per core
N_CORES = 8

_cache = {}


def _build():
    nc = bacc.Bacc("TRN2", target_bir_lowering=False, debug=False,
                   num_devices=N_CORES)

    # ---- DRAM I/O -------------------------------------------------------
    xT_d = nc.dram_tensor("xT", [DIM, N], BF16, kind="ExternalInput")
    wqk_d = nc.dram_tensor("wqk", [DIM, 1024], BF16, kind="ExternalInput")
    wv_d = nc.dram_tensor("wv", [DIM, 512], BF16, kind="ExternalInput")
    wout_d = nc.dram_tensor("wout", [512, DIM], BF16, kind="ExternalInput")
    ident_d = nc.dram_tensor("ident", [128, 128], BF16, kind="ExternalInput")
    maskqk_d = nc.dram_tensor("maskqk", [128, 2080], BF16, kind="ExternalInput")
    mean_d = nc.dram_tensor("meanm", [128, 32], BF16, kind="ExternalInput")
    triexc_d = nc.dram_tensor("triexc", [8, 8], BF16, kind="ExternalInput")
    ones18_d = nc.dram_tensor("ones18", [128, 8], BF16, kind="ExternalInput")
    ones81_d = nc.dram_tensor("ones81", [8, 1], BF16, kind="ExternalInput")
    ones11_d = nc.dram_tensor("ones11", [1, 1], BF16, kind="ExternalInput")
    out_d = nc.dram_tensor("out", [N, DIM], BF16, kind="ExternalOutput")

    with tile.TileContext(nc) as tc:
        with (
            tc.tile_pool(name="const", bufs=1) as cpool,
            tc.tile_pool(name="persist", bufs=1) as ppool,
            tc.tile_pool(name="work", bufs=2) as wpool,
            tc.tile_pool(name="psq", bufs=5, space="PSUM") as psq,
            tc.tile_pool(name="psa", bufs=3, space="PSUM") as psa,
        ):
            # ---- DMA order: block-0 data first so PE starts ASAP --------
            wqk = [ppool.tile([128, 1024], BF16, name=f"wqk{i}", tag=f"wqk{i}")
                   for i in range(8)]
            wv = [ppool.tile([128, 512], BF16, name=f"wv{i}", tag=f"wv{i}")
                  for i in range(8)]
            xT = [ppool.tile([128, N], BF16, name=f"xT{i}", tag=f"xT{i}")
                  for i in range(8)]
            # spread the startup-critical loads over 4 engine queues so the
            # first projection matmuls aren't gated on serial DMA issue
            head_engs = [nc.sync, nc.scalar, nc.gpsimd]
            for i in range(8):
                head_engs[i % 3].dma_start(wqk[i][:, 0:256],
                                           wqk_d[i * 128:(i + 1) * 128, 0:256])
                head_engs[(i + 1) % 3].dma_start(xT[i][:, 0:TB],
                                                 xT_d[i * 128:(i + 1) * 128, 0:TB])
            for i in range(8):
                nc.sync.dma_start(wqk[i][:, 256:1024],
                                  wqk_d[i * 128:(i + 1) * 128, 256:1024])
                nc.sync.dma_start(wv[i][:], wv_d[i * 128:(i + 1) * 128, :])

            ident = cpool.tile([128, 128], BF16, name="ident")
            nc.sync.dma_start(ident[:], ident_d[:])
            maskqk = cpool.tile([128, 32, 65], BF16, name="maskqk")
            nc.sync.dma_start(maskqk[:], maskqk_d.rearrange("p (c k) -> p c k", c=32))
            meanm = cpool.tile([128, 32], BF16, name="meanm")
            nc.sync.dma_start(meanm[:], mean_d[:])
            triexc = cpool.tile([8, 8], BF16, name="triexc")
            nc.sync.dma_start(triexc[:], triexc_d[:])
            ones18 = cpool.tile([128, 8], BF16, name="ones18")
            nc.sync.dma_start(ones18[:], ones18_d[:])
            ones81 = cpool.tile([8, 1], BF16, name="ones81")
            nc.sync.dma_start(ones81[:], ones81_d[:])
            ones11 = cpool.tile([1, 1], BF16, name="ones11")
            nc.sync.dma_start(ones11[:], ones11_d[:])

            wout = []
            for p in range(NPAIR):
                t = ppool.tile([128, 1024], BF16, name=f"wout{p}", tag=f"wout{p}")
                nc.sync.dma_start(t[:], wout_d[p * 128:(p + 1) * 128, :])
                wout.append(t)
            # blocks 1-7 of xT: one big DMA per K-tile
            for i in range(8):
                nc.sync.dma_start(xT[i][:, TB:N], xT_d[i * 128:(i + 1) * 128, TB:N])

            # ---- cross-block running state ------------------------------
            runv = ppool.tile([1, 512], BF16, name="runv", tag="runv")
            nc.vector.memset(runv[:], 0.0)
            runkT = ppool.tile([128, 4], F32, name="runkT", tag="runkT")
            nc.vector.memset(runkT[:], 0.0)

            # ---- staged emission ----------------------------------------
            def stage_a(t, S):
                tok0 = t * TB

                # ---- qkT projection: 8 M-tiles (4 q pairs, 4 k pairs) ----
                qT_all = wpool.tile([128, 4, 8, 64], BF16, name=f"qT_all_{t}",
                                    tag="qT_all")
                kT_all = wpool.tile([128, 4, 8, 65], BF16, name=f"kT_all_{t}",
                                    tag="kT_all")
                # groups interleaved in pairs so each LDWEIGHTS hides behind
                # the other group's stream
                for mt0 in range(0, 8, 2):
                    accs = [psq.tile([128, 512], F32, name=f"qk_ps_{t}_{mt0+j}",
                                     tag="m") for j in range(2)]
                    for i in range(8):
                        for j in range(2):
                            mt = mt0 + j
                            nc.tensor.matmul(
                                accs[j][:], wqk[i][:, mt * 128:(mt + 1) * 128],
                                xT[i][:, tok0:tok0 + TB],
                                start=(i == 0), stop=(i == 7))
                    for j in range(2):
                        mt = mt0 + j
                        dst = (qT_all[:, mt, :, :] if mt < 4
                               else kT_all[:, mt % 4, :, 0:64])
                        src = accs[j][:].rearrange("p (c k) -> p c k", c=8)
                        if mt % 2 == 0:
                            nc.scalar.copy(dst, src)
                        else:
                            nc.vector.tensor_copy(dst, src)

                # ---- cum_kT chain (vector; only needs kT_all cols 0:64) --
                ckT = wpool.tile([128, 4, 8], F32, name=f"ckT_{t}", tag="ckT")
                nc.vector.tensor_reduce(ckT[:], kT_all[:, :, :, 0:64],
                                        axis=mybir.AxisListType.X,
                                        op=ALU.add)
                t1 = wpool.tile([128, 4, 8], F32, name=f"t1_{t}", tag="t1")
                nc.vector.tensor_copy(t1[:, :, 0:1], ckT[:, :, 0:1])
                nc.vector.tensor_add(t1[:, :, 1:8], ckT[:, :, 0:7], ckT[:, :, 1:8])
                t2 = wpool.tile([128, 4, 8], F32, name=f"t2_{t}", tag="t2")
                nc.vector.tensor_copy(t2[:, :, 0:2], t1[:, :, 0:2])
                nc.vector.tensor_add(t2[:, :, 2:8], t1[:, :, 0:6], t1[:, :, 2:8])
                incl = wpool.tile([128, 4, 8], F32, name=f"incl_{t}", tag="incl")
                nc.vector.tensor_copy(incl[:, :, 0:4], t2[:, :, 0:4])
                nc.vector.tensor_add(incl[:, :, 4:8], t2[:, :, 0:4], t2[:, :, 4:8])
                cumkT = wpool.tile([128, 4, 8], F32, name=f"cumkT_{t}", tag="cumkT")
                nc.vector.tensor_copy(cumkT[:, :, 0:1],
                                      runkT[:].broadcast_to([128, 4, 1]))
                nc.vector.tensor_add(cumkT[:, :, 1:8], incl[:, :, 0:7],
                                     runkT[:].broadcast_to([128, 4, 7]))
                nc.vector.tensor_add(runkT[:], runkT[:],
                                     incl[:, :, 7:8].rearrange("p a b -> p (a b)"))
                # 65th k column = cum_k / 64 (chunk mean; scores matmul then
                # gives cross_pre = q . cum_k * SCALE in col 64)
                nc.vector.tensor_scalar_mul(
                    kT_all[:, :, :, 64:65],
                    cumkT[:].rearrange("p a (b o) -> p a b o", o=1), 1.0 / 64)

                # ---- v projection: 4 token tiles -------------------------
                v_sb = [None] * 4
                v_lo = [None] * 4  # odd chunk shifted to partitions 0-63
                chunkv_ps = psq.tile([8, 512], F32, name=f"cv_ps_{t}", tag="m")
                for vt0 in range(0, 4, 2):
                    accs = [psq.tile([128, 512], F32, name=f"v_ps_{t}_{vt0+j}",
                                     tag="m") for j in range(2)]
                    for i in range(8):
                        for j in range(2):
                            vt = vt0 + j
                            nc.tensor.matmul(
                                accs[j][:],
                                xT[i][:, tok0 + vt * 128:tok0 + (vt + 1) * 128],
                                wv[i][:], start=(i == 0), stop=(i == 7))
                    for j in range(2):
                        vt = vt0 + j
                        dst = wpool.tile([128, 512], BF16, name=f"v_{t}_{vt}",
                                         tag=f"v{vt}")
                        nc.vector.tensor_copy(dst[:], accs[j][:])
                        v_sb[vt] = dst
                        dst2 = wpool.tile([64, 512], BF16, name=f"vlo_{t}_{vt}",
                                          tag=f"vlo{vt}")
                        nc.gpsimd.tensor_copy(dst2[:], dst[64:128, :])
                        v_lo[vt] = dst2
                        # chunk means (x 0.5/64 folded into meanm)
                        nc.tensor.matmul(chunkv_ps[:], meanm[:, vt * 8:(vt + 1) * 8],
                                         dst[:], start=(vt == 0), stop=(vt == 3))

                chunkv = wpool.tile([8, 512], BF16, name=f"cv_{t}", tag="cv")
                nc.scalar.copy(chunkv[:], chunkv_ps[:])

                # ---- exclusive cumsum over chunks + running carry -------
                cumv_ps = psq.tile([8, 512], F32, name=f"cumv_ps_{t}", tag="m")
                nc.tensor.matmul(cumv_ps[:], triexc[:], chunkv[:],
                                 start=True, stop=False)
                nc.tensor.matmul(cumv_ps[:], ones18[0:1, :], runv[:],
                                 start=False, stop=True)
                cumv = wpool.tile([8, 512], BF16, name=f"cumv_{t}", tag="cumv")
                nc.scalar.copy(cumv[:], cumv_ps[:])
                # running += sum_c chunk_v
                runp = psq.tile([1, 512], F32, name=f"runp_ps_{t}", tag="m")
                nc.tensor.matmul(runp[:], ones81[:], chunkv[:],
                                 start=True, stop=False)
                nc.tensor.matmul(runp[:], ones11[:], runv[:],
                                 start=False, stop=True)
                nc.scalar.copy(runv[:], runp[:])

                S.update(qT_all=qT_all, kT_all=kT_all, v_sb=v_sb,
                         v_lo=v_lo, cumv=cumv)

            def stage_b1(t, S):
                qT_all, kT_all = S["qT_all"], S["kT_all"]
                # scores (+ cross_pre in col 64) + exp per pair
                E = wpool.tile([128, 32, 65], BF16, name=f"E_{t}", tag="E")
                for p in range(NPAIR):
                    for g in range(2):
                        sx = psa.tile([128, 4, 65], F32, name=f"s_{t}_{p}_{g}",
                                      tag="m")
                        for cc in range(4):
                            c = 4 * g + cc
                            nc.tensor.matmul(
                                sx[0:64, cc, :],
                                qT_all[0:64, p, c, :], kT_all[0:64, p, c, :],
                                start=True, stop=True, tile_position=(0, 0))
                            nc.tensor.matmul(
                                sx[64:128, cc, :],
                                qT_all[64:128, p, c, :], kT_all[64:128, p, c, :],
                                start=True, stop=True, tile_position=(64, 64))
                        nc.scalar.activation(E[:, 8 * p + 4 * g:8 * p + 4 * g + 4, :],
                                             sx[:], AFT.Exp)
                S["E"] = E

            def stage_b2v(t, S):
                E = S["E"]
                # sigmoid(cross_pre) = E64 / (E64 + 1), from exp values
                E64 = E[:, :, 64:65].rearrange("p a o -> p (a o)")
                sigt = wpool.tile([128, 32], F32, name=f"sigt_{t}", tag="sigt")
                nc.vector.tensor_scalar_add(sigt[:], E64, 1.0)
                nc.vector.reciprocal(sigt[:], sigt[:])
                crossQ = wpool.tile([128, 4, 8], BF16, name=f"crq_{t}", tag="crq")
                nc.vector.tensor_mul(
                    crossQ[:].rearrange("p a b -> p (a b)"), E64, sigt[:])
                # mask (zeroes col 64 and causal-upper) then softmax, per
                # pair so attn(p) is ready as early as possible
                denom = wpool.tile([128, 32], F32, name=f"den_{t}", tag="den")
                recip = wpool.tile([128, 32], F32, name=f"rec_{t}", tag="rec")
                attn = wpool.tile([128, 32, 65], BF16, name=f"attn_{t}", tag="attn")
                for p in range(NPAIR):
                    sl = slice(8 * p, 8 * (p + 1))
                    nc.vector.tensor_mul(E[:, sl, :], E[:, sl, :], maskqk[:, sl, :])
                    nc.vector.tensor_reduce(denom[:, sl], E[:, sl, :],
                                            axis=mybir.AxisListType.X, op=ALU.add)
                    nc.vector.reciprocal(recip[:, sl], denom[:, sl])
                    nc.vector.tensor_mul(attn[:, sl, :], E[:, sl, :],
                                         recip[:, sl].broadcast_to([128, 8, 65]))
                S.update(crossQ=crossQ, attn=attn)

            def stage_b2(t, S):
                v_sb, v_lo, cumv = S["v_sb"], S["v_lo"], S["cumv"]
                crossQ, attn = S["crossQ"], S["attn"]

                oTs = []
                etw = {}
                for pp in range(NPAIR + 1):
                    if pp < NPAIR:
                        p = pp
                        # crossT: [8 chunks, 128 (qA|qB)] per pair
                        crt_ps = psa.tile([8, 128], BF16, name=f"crt_ps_{t}_{p}",
                                          tag="m")
                        nc.tensor.transpose(crt_ps[:], crossQ[:, p, :], ident[:])
                        # block-diag: crossT_bd[c, c2, q] = crossT[c, q] * (c==c2)
                        crossT_bd = wpool.tile([8, 8, 128], BF16,
                                               name=f"crb_{t}_{p}",
                                               tag=f"crb{p % 2}")
                        nc.vector.tensor_mul(
                            crossT_bd[:],
                            crt_ps[:].rearrange("p (o q) -> p o q", o=1)
                                .broadcast_to([8, 8, 128]),
                            ident[0:8, 0:8].rearrange("p (c o) -> p c o", o=1)
                                .broadcast_to([8, 8, 128]))
                        # transpose attn per chunk: [128q, 64k] -> [64k, 128q]
                        et1 = psa.tile([64, 512], BF16, name=f"et1_{t}_{p}", tag="m")
                        et2 = psa.tile([64, 512], BF16, name=f"et2_{t}_{p}", tag="m")
                        for c in range(8):
                            dst_ps = et1 if c < 4 else et2
                            nc.tensor.transpose(
                                dst_ps[:, (c % 4) * 128:(c % 4 + 1) * 128],
                                attn[:, 8 * p + c, 0:64], ident[:])
                        ET = wpool.tile([64, 8, 128], BF16, name=f"ET_{t}_{p}",
                                        tag=f"ET{p % 2}")
                        nc.scalar.copy(ET[:, 0:4, :],
                                       et1[:].rearrange("p (c q) -> p c q", c=4))
                        nc.vector.tensor_copy(ET[:, 4:8, :],
                                              et2[:].rearrange("p (c q) -> p c q", c=4))
                        etw[p] = (ET, crossT_bd)

                    if pp >= 1:
                        p = pp - 1
                        ET, crossT_bd = etw.pop(p)
                        # out_localT + cross term, accumulated in PSUM
                        o_ps = psa.tile([128, 512], F32, name=f"o_{t}_{p}", tag="m")
                        for c in range(8):
                            vt_, lo = c // 2, (c % 2)
                            vA = (v_sb[vt_] if lo == 0 else v_lo[vt_])
                            nc.tensor.matmul(
                                o_ps[0:64, c * 64:(c + 1) * 64],
                                vA[0:64, 2 * p * 64:(2 * p + 1) * 64],
                                ET[:, c, 0:64],
                                start=(c == 0), stop=False, tile_position=(0, 0),
                                skip_group_check=True)
                            nc.tensor.matmul(
                                o_ps[64:128, c * 64:(c + 1) * 64],
                                vA[0:64, (2 * p + 1) * 64:(2 * p + 2) * 64],
                                ET[:, c, 64:128],
                                start=(c == 0), stop=False, tile_position=(0, 64),
                                skip_group_check=True)
                        # cross: out[:, c-seg] += cumv[c] (x) sigmoid(cross_pre)
                        nc.tensor.matmul(
                            o_ps[0:64, :].rearrange("p (c q) -> p c q", c=8),
                            cumv[:, 2 * p * 64:(2 * p + 1) * 64],
                            crossT_bd[:, :, 0:64],
                            start=False, stop=True, tile_position=(0, 0),
                            skip_group_check=True)
                        nc.tensor.matmul(
                            o_ps[64:128, :].rearrange("p (c q) -> p c q", c=8),
                            cumv[:, (2 * p + 1) * 64:(2 * p + 2) * 64],
                            crossT_bd[:, :, 64:128],
                            start=False, stop=True, tile_position=(0, 64),
                            skip_group_check=True)
                        oT = wpool.tile([128, 512], BF16, name=f"oT_{t}_{p}",
                                        tag=f"oT{p}")
                        nc.scalar.copy(oT[:], o_ps[:])
                        oTs.append(oT)
                S["oTs"] = oTs

            def stage_c(t, S):
                tok0 = t * TB
                oTs = S["oTs"]
                # ---- out projection (bf16 partial out) ------------------
                groups = [(nt, tt) for nt in range(2) for tt in range(4)]
                for g0 in range(0, 8, 2):
                    fos = [psq.tile([128, 512], F32,
                                    name=f"fo_{t}_{groups[g0+j][0]}_{groups[g0+j][1]}",
                                    tag="m") for j in range(2)]
                    for p in range(NPAIR):
                        for j in range(2):
                            nt, tt = groups[g0 + j]
                            nc.tensor.matmul(
                                fos[j][:], oTs[p][:, tt * 128:(tt + 1) * 128],
                                wout[p][:, nt * 512:(nt + 1) * 512],
                                start=(p == 0), stop=(p == 3))
                    for j in range(2):
                        nt, tt = groups[g0 + j]
                        fs = wpool.tile([128, 512], BF16, name=f"fs_{t}_{nt}_{tt}",
                                        tag="fs")
                        if j == 0:
                            nc.scalar.copy(fs[:], fos[j][:])
                        else:
                            nc.vector.tensor_copy(fs[:], fos[j][:])
                        if t == NBLK - 1:
                            # tail: split the final stores across 3 engine
                            # queues so they drain on parallel DMA engines
                            dst = out_d[tok0 + tt * 128:tok0 + (tt + 1) * 128,
                                        nt * 512:(nt + 1) * 512]
                            nc.sync.dma_start(dst[0:48, :], fs[0:48, :])
                            nc.scalar.dma_start(dst[48:96, :], fs[48:96, :])
                            nc.gpsimd.dma_start(dst[96:128, :], fs[96:128, :])
                        else:
                            nc.sync.dma_start(
                                out_d[tok0 + tt * 128:tok0 + (tt + 1) * 128,
                                      nt * 512:(nt + 1) * 512], fs[:])

            state = {}
            for t in range(NBLK + 2):
                if 1 <= t <= NBLK:
                    stage_b1(t - 1, state[t - 1])
                    stage_b2v(t - 1, state[t - 1])
                if t < NBLK:
                    state[t] = {}
                    stage_a(t, state[t])
                if t >= 2:
                    stage_c(t - 2, state[t - 2])
                if 1 <= t <= NBLK:
                    stage_b2(t - 1, state[t - 1])

    nc.compile()
    return nc


def _consts():
    ident = np.eye(128, dtype=ml_dtypes.bfloat16)
    # causal in-chunk mask with a zero 65th column (cross_pre slot)
    q = np.arange(128)[:, None] % 64
    j = np.arange(64)[None, :]
    m64 = (j <= q).astype(np.float32)                      # [128, 64]
    m65 = np.concatenate([m64, np.zeros((128, 1), np.float32)], axis=1)
    maskqk = np.tile(m65, (1, 32)).astype(ml_dtypes.bfloat16)
    # chunk-mean matrices with 0.5 (cross factor) / 64 (mean) folded in
    meanm = np.zeros((128, 32), dtype=np.float32)
    for vt in range(4):
        meanm[0:64, vt * 8 + 2 * vt] = 0.5 / 64
        meanm[64:128, vt * 8 + 2 * vt + 1] = 0.5 / 64
    triexc = np.triu(np.ones((8, 8), np.float32), 1)  # [c', c] = 1 if c' < c
    ones18 = np.ones((128, 8), np.float32)
    ones81 = np.ones((8, 1), np.float32)
    ones11 = np.ones((1, 1), np.float32)
    bf = ml_dtypes.bfloat16
    return {
        "ident": ident,
        "maskqk": maskqk,
        "meanm": meanm.astype(bf),
        "triexc": triexc.astype(bf),
        "ones18": ones18.astype(bf),
        "ones81": ones81.astype(bf),
        "ones11": ones11.astype(bf),
    }


def _in_maps(x, W_qkv, W_out):
    bf = ml_dtypes.bfloat16
    consts = _consts()
    maps = []
    for c in range(N_CORES):
        b, hh = c // 2, c % 2
        heads = list(range(hh * HPC, (hh + 1) * HPC))
        xT = np.ascontiguousarray(x[b].T).astype(bf)
        qcols = np.concatenate(
            [W_qkv[:, 0 * DIM + h * D:(0 * DIM) + (h + 1) * D] for h in heads], axis=1)
        kcols = np.concatenate(
            [W_qkv[:, 1 * DIM + h * D:1 * DIM + (h + 1) * D] for h in heads], axis=1)
        vcols = np.concatenate(
            [W_qkv[:, 2 * DIM + h * D:2 * DIM + (h + 1) * D] for h in heads], axis=1)
        wqk = np.concatenate([qcols * SCALE, kcols], axis=1).astype(bf)
        wv = vcols.astype(bf)
        wout = np.concatenate([W_out[h * D:(h + 1) * D, :] for h in heads],
                              axis=0).astype(bf)
        m = {"xT": xT, "wqk": np.ascontiguousarray(wqk),
             "wv": np.ascontiguousarray(wv), "wout": np.ascontiguousarray(wout)}
        m.update(consts)
        maps.append(m)
    return maps


def kernel(x, W_qkv, W_out, _trace=False):
    if "nc" not in _cache:
        _cache["nc"] = _build()
    nc = _cache["nc"]
    maps = _in_maps(np.asarray(x, np.float32), np.asarray(W_qkv, np.float32),
                    np.asarray(W_out, np.float32))
    res = run_bass_kernel_spmd(nc, maps, core_ids=list(range(N_CORES)),
                               trace=_trace)
    _cache["last_result"] = res
    out = np.empty((B, N, DIM), np.float32)
    for b in range(B):
        out[b] = (res.results[2 * b]["out"].astype(np.float32)
                  + res.results[2 * b + 1]["out"].astype(np.float32))
    return out


# revision 38
# speedup vs baseline: 1.0259x; 1.0259x over previous
"""ChunkedLinearAttention Trainium2 kernel — 8-core SPMD.

Sharding: core c -> batch b = c//2, head-half hh = c%2 (8 of 16 heads).
Each core computes qkv projection for its heads, chunked local attention +
cross-chunk linear term, and a row-sharded out-projection producing a partial
[4096, 1024] bf16 output; host sums the two half partials per batch element.

All matmuls in bf16 (fp32 accumulate in PSUM).  Layouts:
  xT    [1024, 4096]  x[b] transposed (host-side), bf16
  qkT   [cols, tok]   computed on PE: lhsT=Wqk tile, rhs=xT tile
  v     [tok, vcols]  computed on PE: lhsT=xT tile, rhs=Wv
  per head-pair: qT [128(2 heads x 64 dims), 8 chunks, 64 tok]
                 kT [128, 8 chunks, 65]  (col 64 = exclusive cum_k mean)
  scores S [128(2 heads x 64 q), 4, 65] x2 via per-chunk matmuls packed with
  tile_position; col 64 gives cross_pre = q . cum_k for free.
  sigmoid(cross_pre) is recovered from exp values: E64/(E64+1) on vector.
  out_localT [128(2 heads x 64 dims), 512 tok] accumulated in PSUM; the cross
  term is added with per-chunk rank-1 matmuls cumv[c] (x) crossT[c].

Pipeline (per emission iteration t):
  stage_a(t)   : projections + chunk means + cumsums (k and v)
  stage_b1(t-1): in-chunk score matmuls + exp
  stage_c(t-2) : out projection + store   (PE cover for b2's vector chain)
  stage_b2(t-1): softmax normalize + attn transpose + out_local + cross
"""

import sys

if "/opt/trn_rl_repo" not in sys.path:
    sys.path.insert(0, "/opt/trn_rl_repo")

import numpy as np
import ml_dtypes

import concourse.bacc as bacc
import concourse.tile as tile
import concourse.mybir as mybir
from concourse.bass_utils import run_bass_kernel_spmd

F32 = mybir.dt.float32
BF16 = mybir.dt.bfloat16
AFT = mybir.ActivationFunctionType
ALU = mybir.AluOpType

DIM, H, D, CS = 1024, 16, 64, 64
SCALE = D ** -0.5
B, N = 4, 4096
NBLK, TB = 8, 512          # token blocks
NC_CHUNKS = 8              # chunks per block
HPC = 8                    # heads per core
NPAIR = 4                  # head pairs per core
N_CORES = 8

_cache = {}


def _build():
    nc = bacc.Bacc("TRN2", target_bir_lowering=False, debug=False,
                   num_devices=N_CORES)

    # ---- DRAM I/O -------------------------------------------------------
    xT_d = nc.dram_tensor("xT", [DIM, N], BF16, kind="ExternalInput")
    wqk_d = nc.dram_tensor("wqk", [DIM, 1024], BF16, kind="ExternalInput")
    wv_d = nc.dram_tensor("wv", [DIM, 512], BF16, kind="ExternalInput")
    wout_d = nc.dram_tensor("wout", [512, DIM], BF16, kind="ExternalInput")
    ident_d = nc.dram_tensor("ident", [128, 128], BF16, kind="ExternalInput")
    maskqk_d = nc.dram_tensor("maskqk", [128, 2080], BF16, kind="ExternalInput")
    mean_d = nc.dram_tensor("meanm", [128, 32], BF16, kind="ExternalInput")
    triexc_d = nc.dram_tensor("triexc", [8, 8], BF16, kind="ExternalInput")
    ones18_d = nc.dram_tensor("ones18", [128, 8], BF16, kind="ExternalInput")
    ones81_d = nc.dram_tensor("ones81", [8, 1], BF16, kind="ExternalInput")
    ones11_d = nc.dram_tensor("ones11", [1, 1], BF16, kind="ExternalInput")
    out_d = nc.dram_tensor("out", [N, DIM], BF16, kind="ExternalOutput")

    with tile.TileContext(nc) as tc:
        with (
            tc.tile_pool(name="const", bufs=1) as cpool,
            tc.tile_pool(name="persist", bufs=1) as ppool,
            tc.tile_pool(name="work", bufs=2) as wpool,
            tc.tile_pool(name="psq", bufs=5, space="PSUM") as psq,
            tc.tile_pool(name="psa", bufs=3, space="PSUM") as psa,
        ):
            # ---- DMA order: block-0 data first so PE starts ASAP --------
            wqk = [ppool.tile([128, 1024], BF16, name=f"wqk{i}", tag=f"wqk{i}")
                   for i in range(8)]
            wv = [ppool.tile([128, 512], BF16, name=f"wv{i}", tag=f"wv{i}")
                  for i in range(8)]
            xT = [ppool.tile([128, N], BF16, name=f"xT{i}", tag=f"xT{i}")
                  for i in range(8)]
            # spread the startup-critical loads over 4 engine queues so the
            # first projection matmuls aren't gated on serial DMA issue
            head_engs = [nc.sync, nc.scalar, nc.gpsimd]
            for i in range(8):
                head_engs[i % 3].dma_start(wqk[i][:, 0:256],
                                           wqk_d[i * 128:(i + 1) * 128, 0:256])
                head_engs[(i + 1) % 3].dma_start(xT[i][:, 0:TB],
                                                 xT_d[i * 128:(i + 1) * 128, 0:TB])
            for i in range(8):
                nc.sync.dma_start(wqk[i][:, 256:1024],
                                  wqk_d[i * 128:(i + 1) * 128, 256:1024])
                nc.sync.dma_start(wv[i][:], wv_d[i * 128:(i + 1) * 128, :])

            ident = cpool.tile([128, 128], BF16, name="ident")
            nc.sync.dma_start(ident[:], ident_d[:])
            maskqk = cpool.tile([128, 32, 65], BF16, name="maskqk")
            nc.sync.dma_start(maskqk[:], maskqk_d.rearrange("p (c k) -> p c k", c=32))
            meanm = cpool.tile([128, 32], BF16, name="meanm")
            nc.sync.dma_start(meanm[:], mean_d[:])
            triexc = cpool.tile([8, 8], BF16, name="triexc")
            nc.sync.dma_start(triexc[:], triexc_d[:])
            ones18 = cpool.tile([128, 8], BF16, name="ones18")
            nc.sync.dma_start(ones18[:], ones18_d[:])
            ones81 = cpool.tile([8, 1], BF16, name="ones81")
            nc.sync.dma_start(ones81[:], ones81_d[:])
            ones11 = cpool.tile([1, 1], BF16, name="ones11")
            nc.sync.dma_start(ones11[:], ones11_d[:])

            wout = []
            for p in range(NPAIR):
                t = ppool.tile([128, 1024], BF16, name=f"wout{p}", tag=f"wout{p}")
                nc.sync.dma_start(t[:], wout_d[p * 128:(p + 1) * 128, :])
                wout.append(t)
            # blocks 1-7 of xT: one big DMA per K-tile
            for i in range(8):
                nc.sync.dma_start(xT[i][:, TB:N], xT_d[i * 128:(i + 1) * 128, TB:N])

            # ---- cross-block running state ------------------------------
            runv = ppool.tile([1, 512], BF16, name="runv", tag="runv")
            nc.vector.memset(runv[:], 0.0)
            runkT = ppool.tile([128, 4], F32, name="runkT", tag="runkT")
            nc.vector.memset(runkT[:], 0.0)

            # ---- staged emission ----------------------------------------
            def stage_a(t, S):
                tok0 = t * TB

                # ---- qkT projection: 8 M-tiles (4 q pairs, 4 k pairs) ----
                qT_all = wpool.tile([128, 4, 8, 64], BF16, name=f"qT_all_{t}",
                                    tag="qT_all")
                kT_all = wpool.tile([128, 4, 8, 65], BF16, name=f"kT_all_{t}",
                                    tag="kT_all")
                # groups interleaved in pairs so each LDWEIGHTS hides behind
                # the other group's stream
                for mt0 in range(0, 8, 2):
                    accs = [psq.tile([128, 512], F32, name=f"qk_ps_{t}_{mt0+j}",
                                     tag="m") for j in range(2)]
                    for i in range(8):
                        for j in range(2):
                            mt = mt0 + j
                            nc.tensor.matmul(
                                accs[j][:], wqk[i][:, mt * 128:(mt + 1) * 128],
                                xT[i][:, tok0:tok0 + TB],
                                start=(i == 0), stop=(i == 7))
                    for j in range(2):
                        mt = mt0 + j
                        dst = (qT_all[:, mt, :, :] if mt < 4
                               else kT_all[:, mt % 4, :, 0:64])
                        src = accs[j][:].rearrange("p (c k) -> p c k", c=8)
                        if mt % 2 == 0:
                            nc.scalar.copy(dst, src)
                        else:
                            nc.vector.tensor_copy(dst, src)

                # ---- cum_kT chain (vector; only needs kT_all cols 0:64) --
                ckT = wpool.tile([128, 4, 8], F32, name=f"ckT_{t}", tag="ckT")
                nc.vector.tensor_reduce(ckT[:], kT_all[:, :, :, 0:64],
                                        axis=mybir.AxisListType.X,
                                        op=ALU.add)
                t1 = wpool.tile([128, 4, 8], F32, name=f"t1_{t}", tag="t1")
                nc.vector.tensor_copy(t1[:, :, 0:1], ckT[:, :, 0:1])
                nc.vector.tensor_add(t1[:, :, 1:8], ckT[:, :, 0:7], ckT[:, :, 1:8])
                t2 = wpool.tile([128, 4, 8], F32, name=f"t2_{t}", tag="t2")
                nc.vector.tensor_copy(t2[:, :, 0:2], t1[:, :, 0:2])
                nc.vector.tensor_add(t2[:, :, 2:8], t1[:, :, 0:6], t1[:, :, 2:8])
                incl = wpool.tile([128, 4, 8], F32, name=f"incl_{t}", tag="incl")
                nc.vector.tensor_copy(incl[:, :, 0:4], t2[:, :, 0:4])
                nc.vector.tensor_add(incl[:, :, 4:8], t2[:, :, 0:4], t2[:, :, 4:8])
                cumkT = wpool.tile([128, 4, 8], F32, name=f"cumkT_{t}", tag="cumkT")
                nc.vector.tensor_copy(cumkT[:, :, 0:1],
                                      runkT[:].broadcast_to([128, 4, 1]))
                nc.vector.tensor_add(cumkT[:, :, 1:8], incl[:, :, 0:7],
                                     runkT[:].broadcast_to([128, 4, 7]))
                nc.vector.tensor_add(runkT[:], runkT[:],
                                     incl[:, :, 7:8].rearrange("p a b -> p (a b)"))
                # 65th k column = cum_k / 64 (chunk mean; scores matmul then
                # gives cross_pre = q . cum_k * SCALE in col 64)
                nc.vector.tensor_scalar_mul(
                    kT_all[:, :, :, 64:65],
                    cumkT[:].rearrange("p a (b o) -> p a b o", o=1), 1.0 / 64)

                # ---- v projection: 4 token tiles -------------------------
                v_sb = [None] * 4
                v_lo = [None] * 4  # odd chunk shifted to partitions 0-63
                chunkv_ps = psq.tile([8, 512], F32, name=f"cv_ps_{t}", tag="m")
                for vt0 in range(0, 4, 2):
                    accs = [psq.tile([128, 512], F32, name=f"v_ps_{t}_{vt0+j}",
                                     tag="m") for j in range(2)]
                    for i in range(8):
                        for j in range(2):
                            vt = vt0 + j
                            nc.tensor.matmul(
                                accs[j][:],
                                xT[i][:, tok0 + vt * 128:tok0 + (vt + 1) * 128],
                                wv[i][:], start=(i == 0), stop=(i == 7))
                    for j in range(2):
                        vt = vt0 + j
                        dst = wpool.tile([128, 512], BF16, name=f"v_{t}_{vt}",
                                         tag=f"v{vt}")
                        nc.vector.tensor_copy(dst[:], accs[j][:])
                        v_sb[vt] = dst
                        dst2 = wpool.tile([64, 512], BF16, name=f"vlo_{t}_{vt}",
                                          tag=f"vlo{vt}")
                        nc.gpsimd.tensor_copy(dst2[:], dst[64:128, :])
                        v_lo[vt] = dst2
                        # chunk means (x 0.5/64 folded into meanm)
                        nc.tensor.matmul(chunkv_ps[:], meanm[:, vt * 8:(vt + 1) * 8],
                                         dst[:], start=(vt == 0), stop=(vt == 3))

                chunkv = wpool.tile([8, 512], BF16, name=f"cv_{t}", tag="cv")
                nc.scalar.copy(chunkv[:], chunkv_ps[:])

                # ---- exclusive cumsum over chunks + running carry -------
                cumv_ps = psq.tile([8, 512], F32, name=f"cumv_ps_{t}", tag="m")
                nc.tensor.matmul(cumv_ps[:], triexc[:], chunkv[:],
                                 start=True, stop=False)
                nc.tensor.matmul(cumv_ps[:], ones18[0:1, :], runv[:],
                                 start=False, stop=True)
                cumv = wpool.tile([8, 512], BF16, name=f"cumv_{t}", tag="cumv")
                nc.scalar.copy(cumv[:], cumv_ps[:])
                # running += sum_c chunk_v
                runp = psq.tile([1, 512], F32, name=f"runp_ps_{t}", tag="m")
                nc.tensor.matmul(runp[:], ones81[:], chunkv[:],
                                 start=True, stop=False)
                nc.tensor.matmul(runp[:], ones11[:], runv[:],
                                 start=False, stop=True)
                nc.scalar.copy(runv[:], runp[:])

                S.update(qT_all=qT_all, kT_all=kT_all, v_sb=v_sb,
                         v_lo=v_lo, cumv=cumv)

            def stage_b1(t, S):
                qT_all, kT_all = S["qT_all"], S["kT_all"]
                # scores (+ cross_pre in col 64) + exp per pair
                E = wpool.tile([128, 32, 65], BF16, name=f"E_{t}", tag="E")
                for p in range(NPAIR):
                    for g in range(2):
                        sx = psa.tile([128, 4, 65], F32, name=f"s_{t}_{p}_{g}",
                                      tag="m")
                        for cc in range(4):
                            c = 4 * g + cc
                            nc.tensor.matmul(
                                sx[0:64, cc, :],
                                qT_all[0:64, p, c, :], kT_all[0:64, p, c, :],
                                start=True, stop=True, tile_position=(0, 0))
                            nc.tensor.matmul(
                                sx[64:128, cc, :],
                                qT_all[64:128, p, c, :], kT_all[64:128, p, c, :],
                                start=True, stop=True, tile_position=(64, 64))
                        nc.scalar.activation(E[:, 8 * p + 4 * g:8 * p + 4 * g + 4, :],
                                             sx[:], AFT.Exp)
                S["E"] = E

            def stage_b2v(t, S):
                E = S["E"]
                # sigmoid(cross_pre) = E64 / (E64 + 1), from exp values
                E64 = E[:, :, 64:65].rearrange("p a o -> p (a o)")
                sigt = wpool.tile([128, 32], F32, name=f"sigt_{t}", tag="sigt")
                nc.vector.tensor_scalar_add(sigt[:], E64, 1.0)
                nc.vector.reciprocal(sigt[:], sigt[:])
                crossQ = wpool.tile([128, 4, 8], BF16, name=f"crq_{t}", tag="crq")
                nc.vector.tensor_mul(
                    crossQ[:].rearrange("p a b -> p (a b)"), E64, sigt[:])
                # mask (zeroes col 64 and causal-upper) then softmax, per
                # pair so attn(p) is ready as early as possible
                denom = wpool.tile([128, 32], F32, name=f"den_{t}", tag="den")
                recip = wpool.tile([128, 32], F32, name=f"rec_{t}", tag="rec")
                attn = wpool.tile([128, 32, 65], BF16, name=f"attn_{t}", tag="attn")
                for p in range(NPAIR):
                    sl = slice(8 * p, 8 * (p + 1))
                    nc.vector.tensor_mul(E[:, sl, :], E[:, sl, :], maskqk[:, sl, :])
                    nc.vector.tensor_reduce(denom[:, sl], E[:, sl, :],
                                            axis=mybir.AxisListType.X, op=ALU.add)
                    nc.vector.reciprocal(recip[:, sl], denom[:, sl])
                    nc.vector.tensor_mul(attn[:, sl, :], E[:, sl, :],
                                         recip[:, sl].broadcast_to([128, 8, 65]))
                S.update(crossQ=crossQ, attn=attn)

            def stage_b2(t, S):
                v_sb, v_lo, cumv = S["v_sb"], S["v_lo"], S["cumv"]
                crossQ, attn = S["crossQ"], S["attn"]

                oTs = []
                etw = {}
                for pp in range(NPAIR + 1):
                    if pp < NPAIR:
                        p = pp
                        # crossT: [8 chunks, 128 (qA|qB)] per pair
                        crt_ps = psa.tile([8, 128], BF16, name=f"crt_ps_{t}_{p}",
                                          tag="m")
                        nc.tensor.transpose(crt_ps[:], crossQ[:, p, :], ident[:])
                        # block-diag: crossT_bd[c, c2, q] = crossT[c, q] * (c==c2)
                        crossT_bd = wpool.tile([8, 8, 128], BF16,
                                               name=f"crb_{t}_{p}",
                                               tag=f"crb{p % 2}")
                        nc.vector.tensor_mul(
                            crossT_bd[:],
                            crt_ps[:].rearrange("p (o q) -> p o q", o=1)
                                .broadcast_to([8, 8, 128]),
                            ident[0:8, 0:8].rearrange("p (c o) -> p c o", o=1)
                                .broadcast_to([8, 8, 128]))
                        # transpose attn per chunk: [128q, 64k] -> [64k, 128q]
                        et1 = psa.tile([64, 512], BF16, name=f"et1_{t}_{p}", tag="m")
                        et2 = psa.tile([64, 512], BF16, name=f"et2_{t}_{p}", tag="m")
                        for c in range(8):
                            dst_ps = et1 if c < 4 else et2
                            nc.tensor.transpose(
                                dst_ps[:, (c % 4) * 128:(c % 4 + 1) * 128],
                                attn[:, 8 * p + c, 0:64], ident[:])
                        ET = wpool.tile([64, 8, 128], BF16, name=f"ET_{t}_{p}",
                                        tag=f"ET{p % 2}")
                        nc.scalar.copy(ET[:, 0:4, :],
                                       et1[:].rearrange("p (c q) -> p c q", c=4))
                        nc.scalar.copy(ET[:, 4:8, :],
                                       et2[:].rearrange("p (c q) -> p c q", c=4))
                        etw[p] = (ET, crossT_bd)

                    if pp >= 1:
                        p = pp - 1
                        ET, crossT_bd = etw.pop(p)
                        # out_localT + cross term, accumulated in PSUM
                        o_ps = psa.tile([128, 512], F32, name=f"o_{t}_{p}", tag="m")
                        for c in range(8):
                            vt_, lo = c // 2, (c % 2)
                            vA = (v_sb[vt_] if lo == 0 else v_lo[vt_])
                            nc.tensor.matmul(
                                o_ps[0:64, c * 64:(c + 1) * 64],
                                vA[0:64, 2 * p * 64:(2 * p + 1) * 64],
                                ET[:, c, 0:64],
                                start=(c == 0), stop=False, tile_position=(0, 0),
                                skip_group_check=True)
                            nc.tensor.matmul(
                                o_ps[64:128, c * 64:(c + 1) * 64],
                                vA[0:64, (2 * p + 1) * 64:(2 * p + 2) * 64],
                                ET[:, c, 64:128],
                                start=(c == 0), stop=False, tile_position=(0, 64),
                                skip_group_check=True)
                        # cross: out[:, c-seg] += cumv[c] (x) sigmoid(cross_pre)
                        nc.tensor.matmul(
                            o_ps[0:64, :].rearrange("p (c q) -> p c q", c=8),
                            cumv[:, 2 * p * 64:(2 * p + 1) * 64],
                            crossT_bd[:, :, 0:64],
                            start=False, stop=True, tile_position=(0, 0),
                            skip_group_check=True)
                        nc.tensor.matmul(
                            o_ps[64:128, :].rearrange("p (c q) -> p c q", c=8),
                            cumv[:, (2 * p + 1) * 64:(2 * p + 2) * 64],
                            crossT_bd[:, :, 64:128],
                            start=False, stop=True, tile_position=(0, 64),
                            skip_group_check=True)
                        oT = wpool.tile([128, 512], BF16, name=f"oT_{t}_{p}",
                                        tag=f"oT{p}")
                        nc.scalar.copy(oT[:], o_ps[:])
                        oTs.append(oT)
                S["oTs"] = oTs

            def stage_c(t, S):
                tok0 = t * TB
                oTs = S["oTs"]
                # ---- out projection (bf16 partial out) ------------------
                groups = [(nt, tt) for nt in range(2) for tt in range(4)]
                for g0 in range(0, 8, 2):
                    fos = [psq.tile([128, 512], F32,
                                    name=f"fo_{t}_{groups[g0+j][0]}_{groups[g0+j][1]}",
                                    tag="m") for j in range(2)]
                    for p in range(NPAIR):
                        for j in range(2):
                            nt, tt = groups[g0 + j]
                            nc.tensor.matmul(
                                fos[j][:], oTs[p][:, tt * 128:(tt + 1) * 128],
                                wout[p][:, nt * 512:(nt + 1) * 512],
                                start=(p == 0), stop=(p == 3))
                    for j in range(2):
                        nt, tt = groups[g0 + j]
                        fs = wpool.tile([128, 512], BF16, name=f"fs_{t}_{nt}_{tt}",
                                        tag="fs")
                        if j == 0:
                            nc.scalar.copy(fs[:], fos[j][:])
                        else:
                            nc.vector.tensor_copy(fs[:], fos[j][:])
                        if t == NBLK - 1:
                            # tail: split the final stores across 3 engine
                            # queues so they drain on parallel DMA engines
                            dst = out_d[tok0 + tt * 128:tok0 + (tt + 1) * 128,
                                        nt * 512:(nt + 1) * 512]
                            nc.sync.dma_start(dst[0:48, :], fs[0:48, :])
                            nc.scalar.dma_start(dst[48:96, :], fs[48:96, :])
                            nc.gpsimd.dma_start(dst[96:128, :], fs[96:128, :])
                        else:
                            nc.sync.dma_start(
                                out_d[tok0 + tt * 128:tok0 + (tt + 1) * 128,
                                      nt * 512:(nt + 1) * 512], fs[:])

            state = {}
            for t in range(NBLK + 2):
                if 1 <= t <= NBLK:
                    stage_b1(t - 1, state[t - 1])
                    stage_b2v(t - 1, state[t - 1])
                if t < NBLK:
                    state[t] = {}
                    stage_a(t, state[t])
                if t >= 2:
                    stage_c(t - 2, state[t - 2])
                if 1 <= t <= NBLK:
                    stage_b2(t - 1, state[t - 1])

    nc.compile()
    return nc


def _consts():
    ident = np.eye(128, dtype=ml_dtypes.bfloat16)
    # causal in-chunk mask with a zero 65th column (cross_pre slot)
    q = np.arange(128)[:, None] % 64
    j = np.arange(64)[None, :]
    m64 = (j <= q).astype(np.float32)                      # [128, 64]
    m65 = np.concatenate([m64, np.zeros((128, 1), np.float32)], axis=1)
    maskqk = np.tile(m65, (1, 32)).astype(ml_dtypes.bfloat16)
    # chunk-mean matrices with 0.5 (cross factor) / 64 (mean) folded in
    meanm = np.zeros((128, 32), dtype=np.float32)
    for vt in range(4):
        meanm[0:64, vt * 8 + 2 * vt] = 0.5 / 64
        meanm[64:128, vt * 8 + 2 * vt + 1] = 0.5 / 64
    triexc = np.triu(np.ones((8, 8), np.float32), 1)  # [c', c] = 1 if c' < c
    ones18 = np.ones((128, 8), np.float32)
    ones81 = np.ones((8, 1), np.float32)
    ones11 = np.ones((1, 1), np.float32)
    bf = ml_dtypes.bfloat16
    return {
        "ident": ident,
        "maskqk": maskqk,
        "meanm": meanm.astype(bf),
        "triexc": triexc.astype(bf),
        "ones18": ones18.astype(bf),
        "ones81": ones81.astype(bf),
        "ones11": ones11.astype(bf),
    }


def _in_maps(x, W_qkv, W_out):
    bf = ml_dtypes.bfloat16
    consts = _consts()
    maps = []
    for c in range(N_CORES):
        b, hh = c // 2, c % 2
        heads = list(range(hh * HPC, (hh + 1) * HPC))
        xT = np.ascontiguousarray(x[b].T).astype(bf)
        qcols = np.concatenate(
            [W_qkv[:, 0 * DIM + h * D:(0 * DIM) + (h + 1) * D] for h in heads], axis=1)
        kcols = np.concatenate(
            [W_qkv[:, 1 * DIM + h * D:1 * DIM + (h + 1) * D] for h in heads], axis=1)
        vcols = np.concatenate(
            [W_qkv[:, 2 * DIM + h * D:2 * DIM + (h + 1) * D] for h in heads], axis=1)
        wqk = np.concatenate([qcols * SCALE, kcols], axis=1).astype(bf)
        wv = vcols.astype(bf)
        wout = np.concatenate([W_out[h * D:(h + 1) * D, :] for h in heads],
                              axis=0).astype(bf)
        m = {"xT": xT, "wqk": np.ascontiguousarray(wqk),
             "wv": np.ascontiguousarray(wv), "wout": np.ascontiguousarray(wout)}
        m.update(consts)
        maps.append(m)
    return maps


def kernel(x, W_qkv, W_out, _trace=False):
    if "nc" not in _cache:
        _cache["nc"] = _build()
    nc = _cache["nc"]
    maps = _in_maps(np.asarray(x, np.float32), np.asarray(W_qkv, np.float32),
                    np.asarray(W_out, np.float32))
    res = run_bass_kernel_spmd(nc, maps, core_ids=list(range(N_CORES)),
                               trace=_trace)
    _cache["last_result"] = res
    out = np.empty((B, N, DIM), np.float32)
    for b in range(B):
        out[b] = (res.results[2 * b]["out"].astype(np.float32)
                  + res.results[2 * b + 1]["out"].astype(np.float32))
    return out
